# revision 4
# baseline (speedup 1.0000x reference)
"""Trainium2 Bass kernel for nn_DifferentiableBSpline (Catmull-Rom spline eval).

The reference maps control_points [B, 16, 2] -> trajectory [B, 256, 2] where,
for the fixed schedule (n_cp=16, num_output_points=256), every output point is
a fixed linear combination of the 16 control points of its sample:

    out[b, j, c] = sum_k W[j, k] * cp[b, k, c]

with W[256, 16] folding the Hermite basis, the per-segment t schedule and the
boundary mirroring. On device this is a tiny-K batched matmul, memory bound on
the output traffic (HBM-per-core cap ~358 GB/s).

This version runs the whole device pipeline in fp16 (tolerance is 2e-2;
fp16-quantized inputs + fp32 PSUM accumulation + fp16 output round measure
~8e-4 max rel err), which halves output HBM traffic vs fp32: 8 MB out +
0.5 MB in + 0.125 MB weights per core ~= 8.6 MB -> ~22 us DMA roofline.
The host upcasts the returned fp16 trajectory to fp32.

Device structure (pure data parallel over batch, B_shard = 8192 per core):
  - host pre-arranges each core's shard into the PE stationary (lhsT) layout
    T[32a + kc, g, m] = cp[512 g + 4 m + a, kc] (fp16)
  - per group g of 512 batches: 4 row-tiled single-pass fp16 TensorE matmuls
    (K=32 at partition 32a, M=128, N=512) against a replicated constant
    W2 [128, 512], into two 2-bank PSUM tiles [128, 1024] fp32
  - each psum tile is drained by ONE engine instruction (DVE for banks 0-1,
    ACT for banks 2-3), converting fp32 -> fp16 into a stage tile laid out so
    the output DMA is one flat [128 x 4 KB-contiguous] transfer per group
  - output DMA per group on the Sync HWDGE ring; batch = 512 g + 4 m + a
"""

import numpy as np

import concourse.mybir as mybir
from concourse import bacc
from concourse.tile import TileContext
from concourse.bass_utils import run_bass_kernel_spmd

N_CORES = 8
B_TOTAL = 65536
B_SHARD = B_TOTAL // N_CORES  # 8192
N_CP = 16
T_OUT = 256
GROUP_B = 512
GROUPS = B_SHARD // GROUP_B  # 16


def _spline_weights() -> np.ndarray:
    """W[256, 16]: trajectory[b] = W @ cp[b] (per coordinate)."""
    segments = N_CP - 1
    pps = T_OUT // segments + 1
    seg_list, t_list = [], []
    count = 0
    for i in range(segments):
        if i == segments - 1:
            ts = np.linspace(0.0, 1.0, T_OUT - count)
        else:
            ts = np.linspace(0.0, 1.0, pps)[:-1]
        seg_list.append(np.full(ts.shape, i, dtype=np.int64))
        t_list.append(ts)
        count += len(ts)
    seg = np.concatenate(seg_list)
    t = np.concatenate(t_list).astype(np.float32)
    assert len(seg) == T_OUT

    t2, t3 = t * t, t * t * t
    h00 = 2 * t3 - 3 * t2 + 1
    h10 = t3 - 2 * t2 + t
    h01 = -2 * t3 + 3 * t2
    h11 = t3 - t2

    j = np.arange(T_OUT)
    w_ext = np.zeros((T_OUT, N_CP + 2), dtype=np.float64)
    w_ext[j, seg] += -0.5 * h10
    w_ext[j, seg + 1] += h00 - 0.5 * h11
    w_ext[j, seg + 2] += h01 + 0.5 * h10
    w_ext[j, seg + 3] += 0.5 * h11

    w = w_ext[:, 1:17].copy()
    w[:, 0] += 2 * w_ext[:, 0]
    w[:, 1] -= w_ext[:, 0]
    w[:, 15] += 2 * w_ext[:, 17]
    w[:, 14] -= w_ext[:, 17]
    return w.astype(np.float32)


def _w2rep() -> np.ndarray:
    """[128, 512] fp16: W2[k*2+c, j*2+c] = W[j, k], replicated on 4 row-groups."""
    w = _spline_weights()
    w2 = np.zeros((32, 512), dtype=np.float32)
    jj = np.arange(T_OUT)
    for c in range(2):
        for k in range(N_CP):
            w2[k * 2 + c, jj * 2 + c] = w[jj, k]
    return np.tile(w2, (4, 1)).astype(np.float16)


def _to_lhsT_layout(shard: np.ndarray) -> np.ndarray:
    """[B_SHARD, 16, 2] fp16 -> [128, GROUPS*128] with
    T[32a+kc, g*128+m] = shard[512g + 4m + a, kc]."""
    arr = shard.reshape(GROUPS, 128, 4, N_CP * 2)  # [g, m, a, kc]
    t = arr.transpose(2, 3, 0, 1).reshape(128, GROUPS * 128)
    return np.ascontiguousarray(t)


_W2REP = _w2rep()
_NC_CACHE = None


def _build():
    nc = bacc.Bacc(
        "TRN2", target_bir_lowering=False, debug=False, num_devices=N_CORES
    )
    f32 = mybir.dt.float32
    f16 = mybir.dt.float16
    cpt = nc.dram_tensor(
        "cpt", [128, GROUPS * 128], f16, kind="ExternalInput"
    ).ap()
    w2 = nc.dram_tensor("w2", [128, 512], f16, kind="ExternalInput").ap()
    out = nc.dram_tensor("out", [B_SHARD, T_OUT, 2], f16, kind="ExternalOutput").ap()

    # output of group g: psum partition m at row-group a is batch
    # 512 g + 4 m + a, so per partition the (a, j, c) free dims are one flat
    # 4 KB contiguous run
    out_v = out.rearrange("(g p a) j c -> g p a (j c)", p=128, a=4)

    with TileContext(nc) as tc:
        with (
            tc.tile_pool(name="const", bufs=1) as cpool,
            tc.tile_pool(name="stage", bufs=6) as stg,
            tc.tile_pool(name="psum", bufs=4, space="PSUM") as pp,
        ):
            # Input on both HWDGE rings: the small head chunk (group 0-3) +
            # weights on the Sync ring so group 0 can start ASAP; the bulk
            # (groups 4-15) on the Scalar ring so it transfers concurrently
            # and never queues ahead of the output DMAs on the Sync ring.
            tt = cpool.tile([128, GROUPS * 128], f16)
            w2t = cpool.tile([128, 512], f16)
            nc.sync.dma_start(out=tt[:, : 128 * 4], in_=cpt[:, : 128 * 4])
            nc.sync.dma_start(out=w2t[:], in_=w2[:])
            nc.scalar.dma_start(out=tt[:, 128 * 4 :], in_=cpt[:, 128 * 4 :])
            for g in range(GROUPS):
                stage = stg.tile([128, 4, 512], f16, tag="stage")
                for h in range(2):  # psum halves: h=0 -> a in {0,1}, h=1 -> {2,3}
                    ps = pp.tile([128, 1024], f32, tag="ps")
                    for i in range(2):
                        a = 2 * h + i
                        nc.tensor.matmul(
                            ps[:, 512 * i : 512 * (i + 1)],
                            lhsT=tt[32 * a : 32 * (a + 1), 128 * g : 128 * (g + 1)],
                            rhs=w2t[32 * a : 32 * (a + 1), :],
                            start=True,
                            stop=True,
                            tile_position=(32 * a, 0),
                        )
                    dst = stage[:, 2 * h : 2 * (h + 1), :]
                    if h == 0:
                        nc.vector.tensor_copy(out=dst, in_=ps[:])
                    else:
                        nc.scalar.copy(out=dst, in_=ps[:])
                if g == GROUPS - 1:
                    # split the last group's output so the final DMA (whose
                    # completion receipt gates the fixed epilogue) is smaller
                    # and its first half starts while the second half drains
                    nc.sync.dma_start(out=out_v[g][:, 0:2, :], in_=stage[:, 0:2, :])
                    nc.sync.dma_start(out=out_v[g][:, 2:4, :], in_=stage[:, 2:4, :])
                else:
                    nc.sync.dma_start(out=out_v[g], in_=stage[:])
    nc.compile()
    return nc


def get_nc():
    global _NC_CACHE
    if _NC_CACHE is None:
        _NC_CACHE = _build()
    return _NC_CACHE


def make_in_maps(cp: np.ndarray) -> list[dict]:
    shards = cp.astype(np.float16).reshape(N_CORES, B_SHARD, N_CP, 2)
    return [
        {"cpt": _to_lhsT_layout(shards[i]), "w2": _W2REP} for i in range(N_CORES)
    ]


def kernel(control_points, num_output_points=None, **_unused):
    assert num_output_points is None or int(num_output_points) == T_OUT
    cp = np.ascontiguousarray(np.asarray(control_points, dtype=np.float32))
    assert cp.shape == (B_TOTAL, N_CP, 2), cp.shape

    nc = get_nc()
    in_maps = make_in_maps(cp)
    last_err = None
    for _attempt in range(3):
        try:
            res = run_bass_kernel_spmd(nc, in_maps, core_ids=list(range(N_CORES)))
            break
        except Exception as e:  # transient NRT device errors clear on retry
            last_err = e
    else:
        raise last_err
    return np.concatenate(
        [res.results[i]["out"] for i in range(N_CORES)], axis=0
    ).astype(np.float32)


# revision 9
# speedup vs baseline: 1.0467x; 1.0467x over previous
"""Trainium2 Bass kernel for nn_DifferentiableBSpline (Catmull-Rom spline eval).

The reference maps control_points [B, 16, 2] -> trajectory [B, 256, 2] where,
for the fixed schedule (n_cp=16, num_output_points=256), every output point is
a fixed linear combination of the 16 control points of its sample:

    out[b, j, c] = sum_k W[j, k] * cp[b, k, c]

with W[256, 16] folding the Hermite basis, the per-segment t schedule and the
boundary mirroring. On device this is a tiny-K batched matmul, memory bound on
the output traffic (HBM-per-core cap ~358 GB/s).

This version runs the whole device pipeline in fp16 (tolerance is 2e-2;
fp16-quantized inputs + fp32 PSUM accumulation + fp16 output round measure
~8e-4 max rel err), which halves output HBM traffic vs fp32: 8 MB out +
0.5 MB in + 0.125 MB weights per core ~= 8.6 MB -> ~22 us DMA roofline.
The host upcasts the returned fp16 trajectory to fp32.

Device structure (pure data parallel over batch, B_shard = 8192 per core):
  - host pre-arranges each core's shard into the PE stationary (lhsT) layout
    T[32a + kc, g, m] = cp[512 g + 4 m + a, kc] (fp16)
  - per group g of 512 batches: 4 row-tiled single-pass fp16 TensorE matmuls
    (K=32 at partition 32a, M=128, N=512) against a replicated constant
    W2 [128, 512], into two 2-bank PSUM tiles [128, 1024] fp32
  - each psum tile is drained by ONE engine instruction (DVE for banks 0-1,
    ACT for banks 2-3), converting fp32 -> fp16 into a stage tile laid out so
    the output DMA is one flat [128 x 4 KB-contiguous] transfer per group
  - output DMA per group on the Sync HWDGE ring; batch = 512 g + 4 m + a
"""

import numpy as np

import concourse.mybir as mybir
from concourse import bacc
from concourse.tile import TileContext
from concourse.bass_utils import run_bass_kernel_spmd

N_CORES = 8
B_TOTAL = 65536
B_SHARD = B_TOTAL // N_CORES  # 8192
N_CP = 16
T_OUT = 256
GROUP_B = 512
GROUPS = B_SHARD // GROUP_B  # 16


def _spline_weights() -> np.ndarray:
    """W[256, 16]: trajectory[b] = W @ cp[b] (per coordinate)."""
    segments = N_CP - 1
    pps = T_OUT // segments + 1
    seg_list, t_list = [], []
    count = 0
    for i in range(segments):
        if i == segments - 1:
            ts = np.linspace(0.0, 1.0, T_OUT - count)
        else:
            ts = np.linspace(0.0, 1.0, pps)[:-1]
        seg_list.append(np.full(ts.shape, i, dtype=np.int64))
        t_list.append(ts)
        count += len(ts)
    seg = np.concatenate(seg_list)
    t = np.concatenate(t_list).astype(np.float32)
    assert len(seg) == T_OUT

    t2, t3 = t * t, t * t * t
    h00 = 2 * t3 - 3 * t2 + 1
    h10 = t3 - 2 * t2 + t
    h01 = -2 * t3 + 3 * t2
    h11 = t3 - t2

    j = np.arange(T_OUT)
    w_ext = np.zeros((T_OUT, N_CP + 2), dtype=np.float64)
    w_ext[j, seg] += -0.5 * h10
    w_ext[j, seg + 1] += h00 - 0.5 * h11
    w_ext[j, seg + 2] += h01 + 0.5 * h10
    w_ext[j, seg + 3] += 0.5 * h11

    w = w_ext[:, 1:17].copy()
    w[:, 0] += 2 * w_ext[:, 0]
    w[:, 1] -= w_ext[:, 0]
    w[:, 15] += 2 * w_ext[:, 17]
    w[:, 14] -= w_ext[:, 17]
    return w.astype(np.float32)


def _w2rep() -> np.ndarray:
    """[128, 512] fp16: W2[k*2+c, j*2+c] = W[j, k], replicated on 4 row-groups."""
    w = _spline_weights()
    w2 = np.zeros((32, 512), dtype=np.float32)
    jj = np.arange(T_OUT)
    for c in range(2):
        for k in range(N_CP):
            w2[k * 2 + c, jj * 2 + c] = w[jj, k]
    return np.tile(w2, (4, 1)).astype(np.float16)


def _to_lhsT_layout(shard: np.ndarray) -> np.ndarray:
    """[B_SHARD, 16, 2] fp16 -> [128, GROUPS*128] with
    T[32a+kc, g*128+m] = shard[512g + 4m + a, kc]."""
    arr = shard.reshape(GROUPS, 128, 4, N_CP * 2)  # [g, m, a, kc]
    t = arr.transpose(2, 3, 0, 1).reshape(128, GROUPS * 128)
    return np.ascontiguousarray(t)


_W2REP = _w2rep()
_NC_CACHE = None


def _build():
    nc = bacc.Bacc(
        "TRN2", target_bir_lowering=False, debug=False, num_devices=N_CORES
    )
    f32 = mybir.dt.float32
    f16 = mybir.dt.float16
    # packed input: columns [0:512] = W2 replicated, [512:512+16*128] = lhsT
    # batch data. One tensor so w2 + the head groups arrive with a single
    # DMA completion receipt (the ~1.4 us HBM receipt is on the critical path)
    pk = nc.dram_tensor(
        "pk", [128, 512 + GROUPS * 128], f16, kind="ExternalInput"
    ).ap()
    out = nc.dram_tensor("out", [B_SHARD, T_OUT, 2], f16, kind="ExternalOutput").ap()

    # output of group g: psum partition m at row-group a is batch
    # 512 g + 4 m + a, so per partition the (a, j, c) free dims are one flat
    # 4 KB contiguous run
    out_v = out.rearrange("(g p a) j c -> g p a (j c)", p=128, a=4)

    with TileContext(nc) as tc:
        with (
            tc.tile_pool(name="const", bufs=1) as cpool,
            tc.tile_pool(name="stage", bufs=6) as stg,
            tc.tile_pool(name="psum", bufs=4, space="PSUM") as pp,
        ):
            # head chunk = w2 + groups 0-3 in ONE DMA (one completion
            # receipt), bulk = groups 4-15 behind it on the same Sync ring
            pkt = cpool.tile([128, 512 + GROUPS * 128], f16)
            head_cols = 512 + 128 * 4
            nc.sync.dma_start(out=pkt[:, :head_cols], in_=pk[:, :head_cols])
            nc.sync.dma_start(out=pkt[:, head_cols:], in_=pk[:, head_cols:])
            for g in range(GROUPS):
                stage = stg.tile([128, 4, 512], f16, tag="stage")
                for h in range(2):  # psum halves: h=0 -> a in {0,1}, h=1 -> {2,3}
                    ps = pp.tile([128, 1024], f32, tag="ps")
                    for i in range(2):
                        a = 2 * h + i
                        nc.tensor.matmul(
                            ps[:, 512 * i : 512 * (i + 1)],
                            lhsT=pkt[
                                32 * a : 32 * (a + 1),
                                512 + 128 * g : 512 + 128 * (g + 1),
                            ],
                            rhs=pkt[32 * a : 32 * (a + 1), 0:512],
                            start=True,
                            stop=True,
                            tile_position=(32 * a, 0),
                        )
                    dst = stage[:, 2 * h : 2 * (h + 1), :]
                    if h == 0:
                        nc.vector.tensor_copy(out=dst, in_=ps[:])
                    else:
                        nc.scalar.copy(out=dst, in_=ps[:])
                if g == GROUPS - 1:
                    # split the last group's output so the final DMA (whose
                    # completion receipt gates the fixed epilogue) is smaller
                    # and its first half starts while the second half drains
                    nc.sync.dma_start(out=out_v[g][:, 0:2, :], in_=stage[:, 0:2, :])
                    nc.sync.dma_start(out=out_v[g][:, 2:4, :], in_=stage[:, 2:4, :])
                else:
                    nc.sync.dma_start(out=out_v[g], in_=stage[:])
    nc.compile()
    return nc


def get_nc():
    global _NC_CACHE
    if _NC_CACHE is None:
        _NC_CACHE = _build()
    return _NC_CACHE


def make_in_maps(cp: np.ndarray) -> list[dict]:
    shards = cp.astype(np.float16).reshape(N_CORES, B_SHARD, N_CP, 2)
    return [
        {"pk": np.ascontiguousarray(
            np.concatenate([_W2REP, _to_lhsT_layout(shards[i])], axis=1)
        )}
        for i in range(N_CORES)
    ]


def kernel(control_points, num_output_points=None, **_unused):
    assert num_output_points is None or int(num_output_points) == T_OUT
    cp = np.ascontiguousarray(np.asarray(control_points, dtype=np.float32))
    assert cp.shape == (B_TOTAL, N_CP, 2), cp.shape

    nc = get_nc()
    in_maps = make_in_maps(cp)
    last_err = None
    for _attempt in range(3):
        try:
            res = run_bass_kernel_spmd(nc, in_maps, core_ids=list(range(N_CORES)))
            break
        except Exception as e:  # transient NRT device errors clear on retry
            last_err = e
    else:
        raise last_err
    return np.concatenate(
        [res.results[i]["out"] for i in range(N_CORES)], axis=0
    ).astype(np.float32)
